# revision 42
# baseline (speedup 1.0000x reference)
import numpy as np
import ml_dtypes

EMBED = 256
NUM_HEADS = 8
HEAD_DIM = 32
NUM_GRAPHS = 64
MAX_LEN = 512
EPS = 1e-5
SCALE = float(1.0 / np.sqrt(HEAD_DIM))
N_CORES = 8
G = NUM_GRAPHS // N_CORES
MASK_NEG = -60.0

_CACHE: dict = {}



def _build_program(Ls: tuple, ob_zero: bool, b_zero: bool, ln_triv: bool):
    import concourse.bass as bass
    import concourse.tile as tile
    from concourse import bacc, mybir
    from contextlib import ExitStack

    fp32 = mybir.dt.float32
    bf16 = mybir.dt.bfloat16
    fp8 = mybir.dt.float8e4
    AF = mybir.ActivationFunctionType
    OP = mybir.AluOpType
    DR = mybir.MatmulPerfMode.DoubleRow

    nc = bacc.Bacc()

    xt_d = nc.declare_dram_parameter("xt", [G, 128, 2, MAX_LEN], bf16, isOutput=False)
    xt8_d = nc.declare_dram_parameter("xt8", [G, 128, 2, MAX_LEN], fp8, isOutput=False)
    knegb_d = nc.declare_dram_parameter("knegb", [128, G * 4], fp32, isOutput=False)
    invn_d = nc.declare_dram_parameter("invn", [1, G], fp32, isOutput=False)
    bcorr_d = nc.declare_dram_parameter("bcorr", [1, G], fp32, isOutput=False)
    wqk8_d = nc.declare_dram_parameter("wqk8", [128, 2, 512], fp8, isOutput=False)
    wv8_d = nc.declare_dram_parameter("wv8", [128, 2, 256], fp8, isOutput=False)
    wo8_d = nc.declare_dram_parameter("wo8", [128, 2, 256], fp8, isOutput=False)
    ident_d = nc.declare_dram_parameter("ident", [128, 128], bf16, isOutput=False)
    qkb_d = nc.declare_dram_parameter("qkb", [128, 4], fp32, isOutput=False)
    vb_d = nc.declare_dram_parameter("vb", [1, 256], fp32, isOutput=False)
    ob_d = nc.declare_dram_parameter("ob", [128, 2], fp32, isOutput=False)
    lnw_d = nc.declare_dram_parameter("lnw", [128, 2], fp32, isOutput=False)
    lnb_d = nc.declare_dram_parameter("lnb", [128, 2], fp32, isOutput=False)
    out_d = nc.declare_dram_parameter("out", [G, 128, 2, MAX_LEN], bf16, isOutput=True)

    nkts = [(int(L) + 127) // 128 for L in Ls]

    with tile.TileContext(nc) as tc, ExitStack() as ctx:
        singles = ctx.enter_context(tc.tile_pool(name="singles", bufs=1))
        xpool = ctx.enter_context(tc.tile_pool(name="xpool", bufs=3))
        qkpool = ctx.enter_context(tc.tile_pool(name="qkpool", bufs=2))
        vpool = ctx.enter_context(tc.tile_pool(name="vpool", bufs=2))
        ppool = ctx.enter_context(tc.tile_pool(name="ppool", bufs=6))
        rpool = ctx.enter_context(tc.tile_pool(name="rpool", bufs=3))
        nqpool = ctx.enter_context(tc.tile_pool(name="nqpool", bufs=4))
        cpool = ctx.enter_context(tc.tile_pool(name="cpool", bufs=2))
        hpool = ctx.enter_context(tc.tile_pool(name="hpool", bufs=G + 1))
        sqpool = ctx.enter_context(tc.tile_pool(name="sqpool", bufs=2))
        opool = ctx.enter_context(tc.tile_pool(name="opool", bufs=8))
        tiny = ctx.enter_context(tc.tile_pool(name="tiny", bufs=2))

        ps_sc = ctx.enter_context(tc.tile_pool(name="ps_sc", bufs=2, space="PSUM"))
        ps_cq = ctx.enter_context(tc.tile_pool(name="ps_cq", bufs=2, space="PSUM"))
        ps_dn = ctx.enter_context(tc.tile_pool(name="ps_dn", bufs=1, space="PSUM"))
        ps_ct = ctx.enter_context(tc.tile_pool(name="ps_ct", bufs=1, space="PSUM"))

        dumm = singles.tile([1, 8], fp32)
        nc.vector.memset(dumm, 0.0)
        dumm2 = singles.tile([1, 8], fp32)
        nc.scalar.activation(out=dumm2[:, :], in_=dumm[:, :], func=AF.Exp)

        xT8_0 = xpool.tile([128, 2, MAX_LEN], fp8, tag="xT8", name="xT8p0")
        nc.sync.dma_start(out=xT8_0[:, :, : int(Ls[0])],
                          in_=xt8_d[0, :, :, : int(Ls[0])])
        wqk8 = singles.tile([128, 2, 512], fp8)
        nc.sync.dma_start(out=wqk8, in_=wqk8_d[:, :, :])
        knegb = singles.tile([128, G * 4], fp32)
        nc.sync.dma_start(out=knegb, in_=knegb_d[:, :])
        xT_0 = xpool.tile([128, 2, MAX_LEN], bf16, tag="xT", name="xTp0")
        nc.sync.dma_start(out=xT_0[:, :, : int(Ls[0])],
                          in_=xt_d[0, :, :, : int(Ls[0])])
        wv8 = singles.tile([128, 2, 256], fp8)
        nc.sync.dma_start(out=wv8, in_=wv8_d[:, :, :])


        def emit_late_consts():
            nonlocal wo8, ident, qkb, vbB, ob, lnw, lnb, invn, bcorr
            wo8 = singles.tile([128, 2, 256], fp8)
            nc.sync.dma_start(out=wo8, in_=wo8_d[:, :, :])
            ident = singles.tile([128, 128], bf16)
            nc.sync.dma_start(out=ident, in_=ident_d[:, :])
            invn = singles.tile([1, G], fp32)
            nc.sync.dma_start(out=invn, in_=invn_d[:, :])
            if not b_zero:
                qkb = singles.tile([128, 4], fp32)
                nc.sync.dma_start(out=qkb, in_=qkb_d[:, :])
                vbrow = singles.tile([1, 256], fp32)
                nc.sync.dma_start(out=vbrow, in_=vb_d[:, :])
                vbB = singles.tile([128, 256], fp32)
                nc.gpsimd.partition_broadcast(vbB[:, :], vbrow[:, :],
                                              channels=128)
            if not ob_zero:
                ob = singles.tile([128, 2], fp32)
                nc.sync.dma_start(out=ob, in_=ob_d[:, :])
                bcorr = singles.tile([1, G], fp32)
                nc.sync.dma_start(out=bcorr, in_=bcorr_d[:, :])
            if not ln_triv:
                lnw = singles.tile([128, 2], fp32)
                nc.sync.dma_start(out=lnw, in_=lnw_d[:, :])
                lnb = singles.tile([128, 2], fp32)
                nc.sync.dma_start(out=lnb, in_=lnb_d[:, :])

        wo8 = ident = qkb = vbB = ob = lnw = lnb = invn = bcorr = None
        ones8 = singles.tile([128, 2, 1], fp8)
        nc.vector.memset(ones8, 1.0)
        ones32 = singles.tile([128, 1], fp32)
        nc.vector.memset(ones32, 1.0)
        ones_ls = singles.tile([128, MAX_LEN], fp32)
        nc.gpsimd.memset(ones_ls, 1.0)
        statsAll = singles.tile([128, 4 * G], fp32)

        h_tiles = []
        x_tiles = [None] * G
        pj_tiles = [None] * G

        def emit_dma(s):
            L = int(Ls[s])
            xT = xpool.tile([128, 2, MAX_LEN], bf16, tag="xT", name=f"xT{s}")
            nc.sync.dma_start(out=xT[:, :, :L], in_=xt_d[s, :, :, :L])
            xT8 = xpool.tile([128, 2, MAX_LEN], fp8, tag="xT8", name=f"xT8{s}")
            nc.sync.dma_start(out=xT8[:, :, :L], in_=xt8_d[s, :, :, :L])
            x_tiles[s] = (xT, xT8)

        def emit_proj_qk(s, m):
            L = int(Ls[s])
            _, xT8 = x_tiles[s]
            if m == 0:
                qT = qkpool.tile([128, 2, MAX_LEN], bf16, tag="qT",
                                 name=f"qT{s}")
                kT = qkpool.tile([128, 2, MAX_LEN], bf16, tag="kT",
                                 name=f"kT{s}")
                pj_tiles[s] = [qT, kT, None]
            dst = pj_tiles[s][m // 2]
            qk_ps = ps_ct.tile([128, MAX_LEN], fp32, tag="ct",
                               name=f"qkps{s}_{m}")
            nc.tensor.matmul(
                qk_ps[:, :L],
                wqk8[:, :, bass.ts(m, 128)],
                xT8[:, :, :L],
                start=True, stop=True,
                perf_mode=DR,
                tile_position=(0, 0),
            )
            if b_zero:
                nc.vector.tensor_copy(dst[:, m % 2, :L], qk_ps[:, :L])
            else:
                nc.vector.tensor_scalar_add(
                    out=dst[:, m % 2, :L],
                    in0=qk_ps[:, :L],
                    scalar1=qkb[:, m : m + 1],
                )
            Lpad = 128 * nkts[s]
            if m == 3 and Lpad > L:
                nc.vector.memset(pj_tiles[s][1][:, :, L:Lpad], 0.0)

        def emit_proj_v(s, tpair):
            L = int(Ls[s])
            nkt = nkts[s]
            _, xT8 = x_tiles[s]
            if tpair == 0:
                pj_tiles[s][2] = vpool.tile([128, 4, 256], fp8, tag="v_sb8",
                                            name=f"vsb{s}")
                ks_last = L - 128 * (nkt - 1)
                if ks_last < 128:
                    nc.vector.memset(pj_tiles[s][2][:, nkt - 1, :], 0.0)
            v_sb8 = pj_tiles[s][2]
            nk = min(2, nkt - 2 * tpair)
            v_ps = ps_ct.tile([128, 2, 256], fp32, tag="ct",
                              name=f"vps{s}_{tpair}")
            for i in range(nk):
                kt2 = 2 * tpair + i
                ks = min(128, L - 128 * kt2)
                nc.tensor.matmul(
                    v_ps[:ks, i, :],
                    xT8[:, :, bass.ds(128 * kt2, ks)],
                    wv8[:, :, :],
                    start=True, stop=True,
                    perf_mode=DR,
                    tile_position=(0, 0),
                )
            ks_all = min(128, L - 128 * (2 * tpair + nk - 1))
            if b_zero:
                if ks_all == 128:
                    nc.vector.tensor_copy(
                        v_sb8[:, 2 * tpair : 2 * tpair + nk, :],
                        v_ps[:, :nk, :])
                else:
                    if nk == 2:
                        nc.vector.tensor_copy(
                            v_sb8[:, 2 * tpair, :], v_ps[:, 0, :])
                    nc.vector.tensor_copy(
                        v_sb8[:ks_all, 2 * tpair + nk - 1, :],
                        v_ps[:ks_all, nk - 1, :])
            else:
                for i in range(nk):
                    ks_i = min(128, L - 128 * (2 * tpair + i))
                    nc.vector.tensor_tensor(
                        out=v_sb8[:ks_i, 2 * tpair + i, :],
                        in0=v_ps[:ks_i, i, :],
                        in1=vbB[:ks_i, :],
                        op=OP.add,
                    )

        def emit_proj(s):
            for m in range(4):
                emit_proj_qk(s, m)
            for tpair in range((nkts[s] + 1) // 2):
                emit_proj_v(s, tpair)

        def emit_attn_g2(s, g2, cq_tiles, hooks=None):
            L = int(Ls[s])
            nkt = nkts[s]
            npair = (nkt + 1) // 2
            qT, kT, v_sb8 = pj_tiles[s]
            ph_t = [None, None]
            for kt2 in range(nkt):
                t, half = kt2 // 2, kt2 % 2
                for pr in range(2):
                    sc_t = ps_sc.tile([128, 2, MAX_LEN], fp32, tag="sc",
                                      name=f"sc{s}_{g2}_{kt2}_{pr}")
                    for j in range(2):
                        hidx = 2 * pr + j
                        nc.tensor.matmul(
                            sc_t[:, j, :L],
                            kT[bass.ts(hidx, 32), g2, bass.ds(128 * kt2, 128)],
                            qT[bass.ts(hidx, 32), g2, :L],
                            start=True, stop=True,
                            tile_position=(32 * hidx, 0),
                        )
                    if half == 0:
                        ph_t[pr] = ppool.tile([128, 2, 2, MAX_LEN], fp8,
                                              tag="ph", name=f"ph{s}_{g2}_{t}_{pr}")
                    nc.scalar.activation(
                        out=ph_t[pr][:, half, :, :L],
                        in_=sc_t[:, :, :L],
                        func=AF.Exp,
                        bias=knegb[:, s * 4 + kt2 : s * 4 + kt2 + 1],
                        scale=SCALE,
                    )
                if hooks and kt2 in hooks:
                    hooks[kt2]()
                if half == 1 or kt2 == nkt - 1:
                    last = t == npair - 1
                    cq_ts, dn_t, touched = cq_tiles
                    for pr in range(2):
                        for qb in range(nkt):
                            qs = min(128, L - 128 * qb)
                            cq = cq_ts[qb // 2]
                            for j in range(2):
                                hidx = 2 * pr + j
                                hg = 4 * g2 + hidx
                                stc = qb // 2 not in touched
                                std = "dn" not in touched
                                touched.add(qb // 2)
                                touched.add("dn")
                                if half == 1:
                                    nc.tensor.matmul(
                                        cq[:qs, qb % 2, bass.ts(hg, 32)],
                                        ph_t[pr][:, :, j,
                                                 bass.ds(128 * qb, qs)],
                                        v_sb8[:, 2 * t : 2 * t + 2,
                                              bass.ts(hg, 32)],
                                        start=stc, stop=last,
                                        perf_mode=DR,
                                        tile_position=(0, 0),
                                        skip_group_check=True,
                                    )
                                    nc.tensor.matmul(
                                        dn_t[:qs, qb, hg : hg + 1],
                                        ph_t[pr][:, :, j,
                                                 bass.ds(128 * qb, qs)],
                                        ones8[:, :, :],
                                        start=std, stop=last,
                                        perf_mode=DR,
                                        tile_position=(0, 0),
                                        skip_group_check=True,
                                    )
                                else:
                                    nc.tensor.matmul(
                                        cq[:qs, qb % 2, bass.ts(hg, 32)],
                                        ph_t[pr][:, 0, j,
                                                 bass.ds(128 * qb, qs)],
                                        v_sb8[:, 2 * t, bass.ts(hg, 32)],
                                        start=stc, stop=True,
                                        tile_position=(0, 0),
                                        skip_group_check=True,
                                    )
                                    nc.tensor.matmul(
                                        dn_t[:qs, qb, hg : hg + 1],
                                        ph_t[pr][:, 0, j,
                                                 bass.ds(128 * qb, qs)],
                                        ones8[:, 0, :],
                                        start=std, stop=True,
                                        tile_position=(0, 0),
                                        skip_group_check=True,
                                    )

        fin_state: dict = {}
        ln_tiles: list = [None] * G

        def fin_ln(s):
            st = ps_ct.tile([1, 4], fp32, tag="ct", name=f"st{s}")
            nc.tensor.matmul(st[:, :], ones32[:, :],
                             statsAll[:, 4 * s : 4 * s + 4],
                             start=True, stop=True)
            w = singles.tile([1, 8], fp32, name=f"lnw{s}")
            nc.vector.tensor_copy(w[:, 0:4], st[:, :])
            wv = w[:, 0:4].rearrange("p (a k) -> p a k", k=2)
            nc.vector.tensor_tensor(out=w[:, 4:6], in0=wv[:, :, 0],
                                    in1=wv[:, :, 1], op=OP.add)
            nc.vector.tensor_scalar_mul(out=w[:, 0:2], in0=w[:, 4:6],
                                        scalar1=invn[:, s : s + 1])
            nc.vector.tensor_tensor(out=w[:, 2:3], in0=w[:, 0:1],
                                    in1=w[:, 0:1], op=OP.mult)
            nc.vector.tensor_tensor(out=w[:, 3:4], in0=w[:, 1:2],
                                    in1=w[:, 2:3], op=OP.subtract)
            nc.vector.tensor_scalar_add(out=w[:, 3:4], in0=w[:, 3:4],
                                        scalar1=EPS)
            nc.vector.tensor_scalar(out=w[:, 6:7], in0=w[:, 3:4],
                                    scalar1=-0.5, scalar2=1.5,
                                    op0=OP.mult, op1=OP.add)
            for _ in range(2):
                nc.vector.tensor_tensor(out=w[:, 4:5], in0=w[:, 6:7],
                                        in1=w[:, 6:7], op=OP.mult)
                nc.vector.tensor_tensor(out=w[:, 5:6], in0=w[:, 4:5],
                                        in1=w[:, 3:4], op=OP.mult)
                nc.vector.tensor_scalar(out=w[:, 5:6], in0=w[:, 5:6],
                                        scalar1=-0.5, scalar2=1.5,
                                        op0=OP.mult, op1=OP.add)
                nc.vector.tensor_tensor(out=w[:, 6:7], in0=w[:, 6:7],
                                        in1=w[:, 5:6], op=OP.mult)
            nc.vector.tensor_tensor(out=w[:, 7:8], in0=w[:, 6:7],
                                    in1=w[:, 0:1], op=OP.mult)
            nc.vector.tensor_scalar_mul(out=w[:, 7:8], in0=w[:, 7:8],
                                        scalar1=-1.0)
            sb = singles.tile([128, 2], fp32, name=f"lnsb{s}")
            nc.gpsimd.partition_broadcast(sb[:, :], w[:, 6:8], channels=128)
            ln_tiles[s] = sb

        def fin_norm(s, cq_tiles):
            L = int(Ls[s])
            nkt = nkts[s]
            cq_ts, dn_t, _ = cq_tiles
            nQs = []
            for qb in range(nkt):
                qs = min(128, L - 128 * qb)
                cq = cq_ts[qb // 2]
                recipQ = rpool.tile([128, 8], fp32, tag="recipQ",
                                    name=f"rq{s}_{qb}")
                nc.vector.reciprocal_approx_fast(recipQ[:qs, :],
                                                 dn_t[:qs, qb, :])
                nQ = nqpool.tile([128, 256], bf16, tag="nQ",
                                 name=f"nQ{s}_{qb}")
                nc.vector.tensor_tensor(
                    out=nQ[:qs, :].rearrange("p (h d) -> p h d", d=32),
                    in0=cq[:qs, qb % 2, :].rearrange("p (h d) -> p h d", d=32),
                    in1=recipQ[:qs, :, None].broadcast_to((qs, 8, 32)),
                    op=OP.mult,
                )
                nQs.append(nQ)
            fin_state[s] = nQs

        def fin_transpose(s):
            L = int(Ls[s])
            nkt = nkts[s]
            nQs = fin_state.pop(s)
            ctxT = ps_ct.tile([128, 2, MAX_LEN], bf16, tag="ct",
                              name=f"ctxT{s}")
            for qb in range(nkt):
                qs = min(128, L - 128 * qb)
                for ch in range(2):
                    nc.tensor.transpose(
                        ctxT[:, ch, bass.ds(128 * qb, qs)],
                        nQs[qb][:qs, bass.ts(ch, 128)],
                        ident[:qs, :qs],
                    )
            ctxN8 = cpool.tile([128, 2, MAX_LEN], fp8, tag="ctxN8",
                               name=f"ctxN{s}")
            nc.vector.tensor_copy(ctxN8[:, :, :L], ctxT[:, :, :L])
            fin_state[s] = ctxN8

        def fin_out(s):
            L = int(Ls[s])
            xT, _ = x_tiles[s]
            ctxN8 = fin_state.pop(s)
            h_sb = hpool.tile([128, 2, MAX_LEN], fp32, tag="h", name=f"h{s}")
            h_tiles.append((h_sb, L))
            hsq = sqpool.tile([128, 2, MAX_LEN], fp32, tag="hsq",
                              name=f"hsq{s}")
            for m2 in range(2):
                op_ps = ps_ct.tile([128, MAX_LEN], fp32, tag="ct",
                                   name=f"opps{s}_{m2}")
                nc.tensor.matmul(
                    op_ps[:, :L],
                    wo8[:, :, bass.ts(m2, 128)],
                    ctxN8[:, :, :L],
                    start=True, stop=True,
                    perf_mode=DR,
                    tile_position=(0, 0),
                )
                if ob_zero:
                    nc.vector.tensor_tensor(
                        out=h_sb[:, m2, :L],
                        in0=op_ps[:, :L],
                        in1=xT[:, m2, :L],
                        op=OP.add,
                    )
                else:
                    nc.vector.affine_then_add(
                        out=h_sb[:, m2, :L],
                        in0=op_ps[:, :L],
                        in1=xT[:, m2, :L],
                        scale=ones32[:, :],
                        bias=ob[:, m2 : m2 + 1],
                    )
                nc.vector.affine_mul_reduce(
                    out=hsq[:, m2, :L],
                    accum_out=statsAll[:, 4 * s + m2 : 4 * s + m2 + 1],
                    in0=h_sb[:, m2, :L],
                    in1=ones_ls[:, :L],
                    scale=1.0,
                    bias=0.0,
                )
                nc.vector.affine_mul_reduce(
                    out=hsq[:, m2, :L],
                    accum_out=statsAll[:, 4 * s + 2 + m2 : 4 * s + 2 + m2 + 1],
                    in0=h_sb[:, m2, :L],
                    in1=h_sb[:, m2, :L],
                    scale=1.0,
                    bias=0.0,
                )
            if ln_triv:
                fin_ln(s)

        def emit_proj_fast(s):
            L = int(Ls[s])
            qT = qkpool.tile([128, 2, MAX_LEN], bf16, tag="qT", name=f"qT{s}")
            kT = qkpool.tile([128, 2, MAX_LEN], bf16, tag="kT", name=f"kT{s}")
            pj_tiles[s] = [qT, kT, None]
            _, xT8 = x_tiles[s]
            tiles = {}
            for m in (0, 2, 1, 3):
                half, m2 = m // 2, m % 2
                dst = (qT, kT)[half]
                if half not in tiles:
                    tiles[half] = ps_sc.tile([128, 2, MAX_LEN], fp32,
                                             tag="sc", name=f"qkpsf{s}_{half}")
                qk_ps = tiles[half]
                nc.tensor.matmul(
                    qk_ps[:, m2, :L],
                    wqk8[:, :, bass.ts(m, 128)],
                    xT8[:, :, :L],
                    start=True, stop=True,
                    perf_mode=DR,
                    tile_position=(0, 0),
                )
                if b_zero:
                    if half == 1:
                        nc.scalar.activation(out=dst[:, m2, :L],
                                             in_=qk_ps[:, m2, :L],
                                             func=AF.Copy)
                    else:
                        nc.vector.tensor_copy(dst[:, m2, :L],
                                              qk_ps[:, m2, :L])
                else:
                    nc.vector.tensor_scalar_add(
                        out=dst[:, m2, :L],
                        in0=qk_ps[:, m2, :L],
                        scalar1=qkb[:, m : m + 1],
                    )
            Lpad = 128 * nkts[s]
            if Lpad > L:
                nc.vector.memset(kT[:, :, L:Lpad], 0.0)
            for tpair in range((nkts[s] + 1) // 2):
                emit_proj_v(s, tpair)

        x_tiles[0] = (xT_0, xT8_0)
        emit_proj_fast(0)
        emit_late_consts()
        if G > 1:
            emit_dma(1)

        prev = [None, None]

        for s in range(G):
            nkt = nkts[s]
            cq_tiles = (
                [ps_cq.tile([128, 2, 256], fp32, tag="cq", name=f"cq{s}_{i}")
                 for i in range((nkt + 1) // 2)],
                ps_dn.tile([128, 4, 8], fp32, tag="dn", name=f"dn{s}"),
                set(),
            )
            cq_tiles[2].clear()

            ps, pcq = prev
            hooks0 = {}
            if ps is not None:
                hooks0[0] = lambda: fin_norm(ps, pcq)
                hooks0[2] = lambda: fin_transpose(ps)

            hooks1 = {}
            if ps is not None:
                hooks1[0] = lambda: fin_out(ps)
            if s + 1 < G:
                def hook_qk():
                    for m in range(4):
                        emit_proj_qk(s + 1, m)

                def hook_v():
                    for tpair in range((nkts[s + 1] + 1) // 2):
                        emit_proj_v(s + 1, tpair)
                    if s + 2 < G:
                        emit_dma(s + 2)
                hooks0[1] = hook_qk
                hooks1[1] = hook_v

            emit_attn_g2(s, 0, cq_tiles, hooks=hooks0)
            emit_attn_g2(s, 1, cq_tiles, hooks=hooks1)
            prev = [s, cq_tiles]

        fin_norm(G - 1, prev[1])
        fin_transpose(G - 1)
        fin_out(G - 1)

        if ln_triv:
            for s in range(G):
                h_sb, L = h_tiles[s]
                sb = ln_tiles[s]
                fin = opool.tile([128, 2, MAX_LEN], bf16, tag="fin",
                                 name=f"fin{s}")
                if s == G - 1:
                    for m2 in range(2):
                        nc.scalar.activation(
                            out=fin[:, m2, :L],
                            in_=h_sb[:, m2, :L],
                            func=AF.Gelu,
                            bias=sb[:, 1:2],
                            scale=sb[:, 0:1],
                        )
                        eng = (nc.sync, nc.gpsimd)[m2]
                        eng.dma_start(out=out_d[s, :, m2, :L],
                                      in_=fin[:, m2, :L])
                else:
                    nc.scalar.activation(
                        out=fin[:, :, :L],
                        in_=h_sb[:, :, :L],
                        func=AF.Gelu,
                        bias=sb[:, 1:2],
                        scale=sb[:, 0:1],
                    )
                    eng = (nc.sync, nc.gpsimd)[s % 2]
                    eng.dma_start(out=out_d[s, :, :, :L], in_=fin[:, :, :L])
            nc.compile()
            return nc

        st_ps = ps_ct.tile([1, 4 * G], fp32, tag="ct")
        nc.tensor.matmul(st_ps[:, :], ones32[:, :], statsAll[:, :],
                         start=True, stop=True)
        statsv = tiny.tile([1, 4 * G], fp32, tag="statsv")
        nc.vector.tensor_copy(statsv[:, :], st_ps[:, :])
        sv = statsv[:, :].rearrange("p (g k) -> p g k", k=4)
        Ssum = tiny.tile([1, G], fp32, tag="Ssum")
        Qsum = tiny.tile([1, G], fp32, tag="Qsum")
        nc.vector.tensor_tensor(out=Ssum[:, :], in0=sv[:, :, 0], in1=sv[:, :, 1],
                                op=OP.add)
        nc.vector.tensor_tensor(out=Qsum[:, :], in0=sv[:, :, 2], in1=sv[:, :, 3],
                                op=OP.add)
        mean = tiny.tile([1, G], fp32, tag="mean")
        nc.vector.tensor_tensor(out=mean[:, :], in0=Ssum[:, :], in1=invn[:, :],
                                op=OP.mult)
        if not ob_zero:
            nc.vector.tensor_tensor(out=mean[:, :], in0=mean[:, :],
                                    in1=bcorr[:, :], op=OP.add)
        ex2 = tiny.tile([1, G], fp32, tag="ex2")
        nc.vector.tensor_tensor(out=ex2[:, :], in0=Qsum[:, :], in1=invn[:, :],
                                op=OP.mult)
        msq = tiny.tile([1, G], fp32, tag="msq")
        nc.vector.tensor_tensor(out=msq[:, :], in0=mean[:, :], in1=mean[:, :],
                                op=OP.mult)
        var = tiny.tile([1, G], fp32, tag="var")
        nc.vector.tensor_tensor(out=var[:, :], in0=ex2[:, :], in1=msq[:, :],
                                op=OP.subtract)
        nc.vector.tensor_scalar_add(out=var[:, :], in0=var[:, :], scalar1=EPS)
        std = tiny.tile([1, G], fp32, tag="std")
        nc.scalar.activation(out=std[:, :], in_=var[:, :], func=AF.Sqrt)
        rstd = tiny.tile([1, G], fp32, tag="rstd")
        nc.vector.reciprocal(out=rstd[:, :], in_=std[:, :])

        rstdB = tiny.tile([128, G], fp32, tag="rstdB")
        nc.gpsimd.partition_broadcast(rstdB[:, :], rstd[:, :], channels=128)
        meanB = tiny.tile([128, G], fp32, tag="meanB")
        nc.gpsimd.partition_broadcast(meanB[:, :], mean[:, :], channels=128)

        if ln_triv:
            biG = tiny.tile([128, G], fp32, tag="biG")
            nc.vector.tensor_tensor(out=biG[:, :], in0=rstdB[:, :],
                                    in1=meanB[:, :], op=OP.mult)
            nc.vector.tensor_scalar_mul(out=biG[:, :], in0=biG[:, :],
                                        scalar1=-1.0)
            for s in range(G):
                h_sb, L = h_tiles[s]
                fin = opool.tile([128, 2, MAX_LEN], bf16, tag="fin",
                                 name=f"fin{s}")
                nc.scalar.activation(
                    out=fin[:, :, :L],
                    in_=h_sb[:, :, :L],
                    func=AF.Gelu,
                    bias=biG[:, s : s + 1],
                    scale=rstdB[:, s : s + 1],
                )
                eng = (nc.sync, nc.gpsimd)[s % 2]
                eng.dma_start(out=out_d[s, :, :, :L], in_=fin[:, :, :L])
        else:
            gsc = []
            for m2 in range(2):
                sc_m = tiny.tile([128, G], fp32, tag=f"sc{m2}")
                nc.vector.tensor_scalar_mul(out=sc_m[:, :], in0=rstdB[:, :],
                                            scalar1=lnw[:, m2 : m2 + 1])
                t1 = tiny.tile([128, G], fp32, tag=f"t1{m2}")
                nc.vector.tensor_scalar(
                    out=t1[:, :], in0=meanB[:, :], scalar1=ob[:, m2 : m2 + 1],
                    scalar2=None, op0=OP.subtract,
                )
                t2 = tiny.tile([128, G], fp32, tag=f"t2{m2}")
                nc.vector.tensor_tensor(out=t2[:, :], in0=sc_m[:, :],
                                        in1=t1[:, :], op=OP.mult)
                bi_m = tiny.tile([128, G], fp32, tag=f"bi{m2}")
                nc.vector.tensor_scalar(
                    out=bi_m[:, :], in0=t2[:, :], scalar1=-1.0,
                    scalar2=lnb[:, m2 : m2 + 1],
                    op0=OP.mult, op1=OP.add,
                )
                gsc.append((sc_m, bi_m))

            for s in range(G):
                h_sb, L = h_tiles[s]
                fin = opool.tile([128, 2, MAX_LEN], bf16, tag="fin",
                                 name=f"fin{s}")
                for m2 in range(2):
                    sc_m, bi_m = gsc[m2]
                    nc.scalar.activation(
                        out=fin[:, m2, :L],
                        in_=h_sb[:, m2, :L],
                        func=AF.Gelu,
                        bias=bi_m[:, s : s + 1],
                        scale=sc_m[:, s : s + 1],
                    )
                nc.sync.dma_start(out=out_d[s, :, :, :L], in_=fin[:, :, :L])

    nc.compile()
    return nc



def kernel(x, batch, in_proj_w, in_proj_b, out_proj_w, out_proj_b,
           ln_weight, ln_bias):
    fp8 = ml_dtypes.float8_e4m3
    x = np.asarray(x, dtype=np.float32)
    batch = np.asarray(batch, dtype=np.int32)
    in_proj_w = np.asarray(in_proj_w, dtype=np.float32)
    in_proj_b = np.asarray(in_proj_b, dtype=np.float32)
    out_proj_w = np.asarray(out_proj_w, dtype=np.float32)
    out_proj_b = np.asarray(out_proj_b, dtype=np.float32)
    ln_weight = np.asarray(ln_weight, dtype=np.float32)
    ln_bias = np.asarray(ln_bias, dtype=np.float32)

    N = x.shape[0]
    counts = np.bincount(batch, minlength=NUM_GRAPHS).astype(np.int64)
    starts = np.concatenate([[0], np.cumsum(counts)[:-1]])

    order = np.argsort(-counts, kind="stable")
    assign = np.empty((N_CORES, G), dtype=np.int64)
    Ls = np.empty(G, dtype=np.int64)
    for s in range(G):
        ranks = order[s * N_CORES : (s + 1) * N_CORES]
        assign[:, s] = ranks
        Ls[s] = min(MAX_LEN, -4 * (-int(counts[ranks].max()) // 4))

    ob_zero = not np.any(out_proj_b != 0.0)
    b_zero = not np.any(in_proj_b != 0.0)
    ln_triv = (ob_zero and np.all(ln_weight == 1.0)
               and not np.any(ln_bias != 0.0))
    key = (tuple(int(v) for v in Ls), ob_zero, b_zero, ln_triv)
    if key not in _CACHE:
        _CACHE[key] = _build_program(key[0], ob_zero, b_zero, ln_triv)
    nc = _CACHE[key]

    Wqk = in_proj_w[:512]
    wqk8 = np.ascontiguousarray(
        Wqk.reshape(4, 128, 2, 128).transpose(3, 2, 0, 1).reshape(128, 2, 512)
    ).astype(fp8)
    Wv = in_proj_w[512:768]
    wv8 = np.ascontiguousarray(
        Wv.reshape(256, 2, 128).transpose(2, 1, 0)).astype(fp8)
    wo8 = np.ascontiguousarray(
        out_proj_w.reshape(256, 2, 128).transpose(2, 1, 0)).astype(fp8)
    ident = np.eye(128, dtype=ml_dtypes.bfloat16)
    qkb = np.ascontiguousarray(in_proj_b[:512].reshape(4, 128).T)
    vb = np.ascontiguousarray(in_proj_b[512:768][None, :])
    ob = np.ascontiguousarray(out_proj_b.reshape(2, 128).T)
    lnw = np.ascontiguousarray(ln_weight.reshape(2, 128).T)
    lnb = np.ascontiguousarray(ln_bias.reshape(2, 128).T)
    sum_ob = float(out_proj_b.sum())

    in_maps = []
    for c in range(N_CORES):
        xt = np.zeros((G, 128, 2, MAX_LEN), dtype=np.float32)
        knegb = np.zeros((128, G * 4), dtype=np.float32)
        invn = np.zeros((1, G), dtype=np.float32)
        bcorr = np.zeros((1, G), dtype=np.float32)
        for s in range(G):
            g = assign[c, s]
            L = int(counts[g])
            xg = x[starts[g] : starts[g] + L]
            xT = xg.T.reshape(2, 128, L).transpose(1, 0, 2)
            xt[s, :, :, :L] = xT
            nkt = (int(Ls[s]) + 127) // 128
            for kt2 in range(nkt):
                pvalid = np.arange(128) + 128 * kt2 < L
                knegb[:, s * 4 + kt2] = np.where(pvalid, 0.0, MASK_NEG)
            invn[0, s] = 1.0 / (L * EMBED)
            bcorr[0, s] = sum_ob * L * invn[0, s]
        in_maps.append(dict(
            xt=xt.astype(ml_dtypes.bfloat16), xt8=xt.astype(fp8),
            knegb=knegb, invn=invn, bcorr=bcorr,
            wqk8=wqk8, wv8=wv8, wo8=wo8, ident=ident, qkb=qkb, vb=vb, ob=ob,
            lnw=lnw, lnb=lnb,
        ))

    from concourse.bass_utils import run_bass_kernel_spmd
    res = run_bass_kernel_spmd(nc, in_maps, list(range(N_CORES)))

    out = np.empty((N, EMBED), dtype=np.float32)
    for c in range(N_CORES):
        o = np.asarray(res.results[c]["out"], dtype=np.float32)
        for s in range(G):
            g = assign[c, s]
            L = int(counts[g])
            outT = o[s, :, :, :L].transpose(1, 0, 2).reshape(EMBED, L)
            out[starts[g] : starts[g] + L] = outT.T
    return out
